# revision 34
# baseline (speedup 1.0000x reference)
"""Trainium2 Bass kernel for nn_LowFreqPenaltyLoss.

Computes mean(|einsum('ih,nchw,jw->ncij', Ch, delta, Cw)|) for
delta [256, 3, 256, 256] f32, Ch/Cw the 8x256 unnormalized DCT-II bases.

Strategy (data-parallel over batch, 8 cores), per core 96 images = 24 MiB:

  LOAD (the problem is HBM-stream-bound): 11 groups of 8 images (2 MiB)
  + tail groups of 4/2/2 images, all via SWDGE DMAs that cast f32->bf16
  inline.  Partition p receives a CONTIGUOUS 16 KiB HBM chunk (group rows
  16p..16p+15), so descriptors/packets are full-sized AND 16 KiB-aligned:
  the old per-row 1 KiB gather streamed at ~275 GB/s read-side, flat
  aligned chunks reach ~400-410 GB/s (17 KiB unaligned strides drop to
  ~310).  All DMAs are issued upfront (all tiles resident, ~100 KiB/part).
  The small tail groups shorten the post-stream serial chain.

  STAGE A (contract h): each partition's rows sit on the free axis, so the
  DCT-H contraction is 16 accumulating matmuls with block-diagonal weights
  wa17[p, r, 8q+i] = Ch[i, h] for global row 16p+r = 256q+h ->
  psumA[8q+i, w] f32 (one [64,256] accumulation per group).

  STAGE B (contract w): ACT copies psumA -> SBUF in two [.,128] halves
  (casts bf16; the halves let the first PE transpose start early), 2 PE
  transposes (each into its own PSUM bank: transpose-mode output must
  start at a bank boundary), DVE copies out, matmul with CwT ->
  ps2[j, (q,i)], fused |.|+sum on DVE into acc[8,1].  Stage B of group g-1
  is emitted AFTER stage A of group g so the PE (which runs in program
  order) never stalls mid-stream on ACT/DVE round-trips.

  FINISH: the tail groups (4/2/2 images) share one tS (free-axis offsets)
  and ONE combined stage-B back-half; acc (main groups) ships early via a
  Sync HWDGE DMA while the tails finish, and the tails' unaccumulated
  reduction ships separately, so the final out-DMA depends only on the last
  DVE reduce.  The host sums 8 cores x 16 partials and divides by 49152.
  bf16 inputs + f32 PSUM accumulation give ~2e-4 relative error (gate 2e-2).

  Known limit (measured this session): SDMA engine 15 sustains only ~21
  GB/s read vs ~26 for engines 0-14 -- under BOTH SWDGE and HWDGE, so it
  is not the SWDGE descriptor-ring theory; it is saturated end-to-end and
  sets the stream length (~74 us busy + ~8 us start + ~6 us tail/exit
  = ~88 us).  Shedding its load was tried and does NOT work:
  full-128-partition DMAs split descriptors evenly over all 16 engines
  (partition-pinned), while PARTIAL-partition DMAs assign descriptors by
  position, not partition -- SWDGE sprays them quasi-randomly (head-of-
  line collapse, 149 us), and HWDGE splits over the largest divisor of
  the partition count <= 16, starting at engine 0 (92 -> engines 0-3 x23;
  96 -> all 16 x6; 28 -> engines 0-13 x2).  Routing 6 images through such
  HWDGE partials off engine 15 worked mechanically (engine 15 dropped to
  ~70 us busy) but a ~5-10%% mixed-queue throughput penalty on the spray
  engines ate the gain (89-92 us).  An all-HWDGE f32 stream with on-chip
  casts also loses (102 us: slower dispatch start, serialized post-stream
  cast chains, and a ~7 us end-of-NEFF event-sem clear parade).  The
  uniform SWDGE layout below is at that layout's floor.
"""

import sys
import types

for _p in ("/root/.axon_site/_ro/trn_rl_repo", "/opt/trn_rl_repo"):
    if _p not in sys.path:
        sys.path.append(_p)

import numpy as np
from contextlib import ExitStack

import concourse.bass as bass
import concourse.tile as tile
from concourse import mybir, bass_utils
from concourse._compat import with_exitstack
from concourse.vector_clock import ScopedClock

# ---------------------------------------------------------------------------
# Workarounds for this image.
# ---------------------------------------------------------------------------

# walrus on this image rejects >1 sync-wait on one CTRL instruction; split the
# Tile exit-drain's waits across follow-up nops (same engine, program order).
# Also: the stock tail (barrier + per-sem clear + barrier) costs ~8-10us of
# EVSEM butterfly at kernel end. The kernel is one-shot per NEFF execution and
# NRT re-initialises semaphores per execution, so keep only the drain + DMA
# completion waits.
_ORIG_DAB = tile.TileContext._drain_and_barrier
_USE_STOCK_TAIL = False


def _patched_drain_and_barrier(self, tick_clock, wait_clock):
    if _USE_STOCK_TAIL:
        return _ORIG_DAB(self, tick_clock, wait_clock)
    nc = self.nc
    drain_inst = nc.sync.drain()
    wait_clock.add_sem_waits(
        drain_inst.ins, ScopedClock({None: tick_clock.global_clock})
    )
    si = drain_inst.ins.sync_info
    waits = list(si.on_wait) if si and si.on_wait else []
    if len(waits) > 1:
        drain_inst.ins.sync_info = mybir.SyncInfo(
            on_wait=[waits[0]], on_update=list(si.on_update or [])
        )
        for w in waits[1:]:
            nop = nc.sync.nop(nofuse=True, hint="drain_wait_split")
            nop.ins.sync_info = mybir.SyncInfo(on_wait=[w], on_update=[])
    popped = nc._tile_sem_poison_stack.pop()
    assert popped is self._sem_poison


tile.TileContext._drain_and_barrier = _patched_drain_and_barrier

# zero-egress container: profiling artifact upload must stay local.
bass_utils.upload_artifacts = lambda d: d


def _strip_main_barrier(nc):
    """Drop the prologue all-engine barrier AND the dead const memsets in
    'main': the barrier's only role is to fence the framework preamble (dead
    const memsets + per-engine table loads) from the kernel, but per-engine
    program order already covers the table loads, and nothing reads the
    const tiles (verified: no instruction references const-* memrefs).  Each
    engine then branches into the kernel as soon as its own init finishes
    instead of waiting for the slowest engine (~1.4us of startup)."""
    for fn in nc.m.functions:
        for bb in fn.blocks:
            if bb.name != "main":
                continue
            bb.instructions[:] = [
                i for i in bb.instructions
                if not isinstance(
                    i,
                    (mybir.InstEventSemaphore, mybir.InstDrain, mybir.InstMemset),
                )
            ]


def _split_multi_waits(nc):
    """walrus on this image rejects >1 sync-wait per instruction: hoist extra
    waits onto fresh NoOps inserted just before, on the same engine."""
    for fn in nc.m.functions:
        for bb in fn.blocks:
            new_insts = []
            for inst in bb.instructions:
                si = inst.sync_info
                waits = list(si.on_wait) if si and si.on_wait else []
                if len(waits) > 1:
                    for w in waits[:-1]:
                        nop = mybir.InstNoOp(
                            name=nc.get_next_instruction_name(),
                            sync_info=mybir.SyncInfo(on_wait=[w], on_update=[]),
                            bass_nofuse=True,
                            engine=inst.engine,
                        )
                        new_insts.append(nop)
                    inst.sync_info = mybir.SyncInfo(
                        on_wait=[waits[-1]], on_update=list(si.on_update or [])
                    )
                new_insts.append(inst)
            bb.instructions[:] = new_insts

# ---------------------------------------------------------------------------
# Problem constants (hardcoded; kernel.py must be self-contained).
# ---------------------------------------------------------------------------

B, C, H, W = 256, 3, 256, 256
LOW_A = LOW_B = 8
N_CORES = 8
IMGS_PER_CORE = (B // N_CORES) * C          # 96
P_IMG = 2                                   # images via the HWDGE mini-spray
GRP = 8                                     # images per main group (2 MiB f32)
N_MAIN = 11                                 # 11x8 main + 2 P + 4/1/1 tails
SHED_ROWS = 16                              # rows per partition (16 KiB chunks)
TOTAL_LOW = B * C * LOW_A * LOW_B           # 49152 -> mean divisor

F32 = mybir.dt.float32
BF16 = mybir.dt.bfloat16


def _dct_basis(K, N):
    n = np.arange(N, dtype=np.float64)
    k = np.arange(K, dtype=np.float64)
    return (2.0 * np.cos(np.pi * (2.0 * n[None, :] + 1.0) * k[:, None] / (2.0 * N))).astype(
        np.float32
    )


def _make_consts():
    Ch = _dct_basis(LOW_A, H)   # [8, 256]
    Cw = _dct_basis(LOW_B, W)   # [8, 256]
    # Main-group weights: partition p holds group rows 16p..16p+15; global
    # row g = 256q + h -> out row 8q+i gets Ch[i, h] (block-diagonal).
    wa17 = np.zeros((128, SHED_ROWS, 64), np.float32)
    for p in range(128):
        for r in range(SHED_ROWS):
            g = SHED_ROWS * p + r
            q, h = divmod(g, H)
            wa17[p, r, 8 * q:8 * q + 8] = Ch[:, h]
    # 4-image tail group: image q = p//32, rows h = 8*(p%32) + r.
    wa4 = np.zeros((128, 8, 32), np.float32)
    for p in range(128):
        q, pp = p // 32, p % 32
        for r in range(8):
            wa4[p, r, 8 * q:8 * q + 8] = Ch[:, 8 * pp + r]
    # 1-image tail groups: rows h = 2p + r.
    wa1 = np.zeros((128, 2, 8), np.float32)
    for p in range(128):
        for r in range(2):
            wa1[p, r, 0:8] = Ch[:, 2 * p + r]
    # P (mini-spray) group: partitions 0..31 hold rows 16p..16p+15 of the
    # first 2 images (512 rows); partitions 32..127 carry zero weights.
    wap = np.zeros((128, SHED_ROWS, 8 * P_IMG), np.float32)
    for p in range(32):
        for r in range(SHED_ROWS):
            q, h = divmod(SHED_ROWS * p + r, H)
            wap[p, r, 8 * q:8 * q + 8] = Ch[:, h]
    # cwt[p, wc, j] = Cw[j, wc*128+p]
    cwt = np.zeros((128, 2, LOW_B), np.float32)
    for wc in range(2):
        cwt[:, wc, :] = Cw[:, wc * 128:(wc + 1) * 128].T
    import ml_dtypes
    bf16 = ml_dtypes.bfloat16
    ident = np.eye(128, dtype=np.float32)
    blob = np.concatenate([
        wa17.reshape(128, -1), wa4.reshape(128, -1), wa1.reshape(128, -1),
        wap.reshape(128, -1), cwt.reshape(128, -1), ident,
    ], axis=1)
    return np.ascontiguousarray(blob).astype(bf16)


CONSTS = _make_consts()


# ---------------------------------------------------------------------------
# Kernel body (per core; SPMD over 8 cores).
# ---------------------------------------------------------------------------

@with_exitstack
def _lowfreq_kernel(ctx: ExitStack, tc, out_ap, delta_ap, consts_ap):
    nc = tc.nc

    const_pool = ctx.enter_context(tc.tile_pool(name="const", bufs=1))
    in8_pool = ctx.enter_context(tc.tile_pool(name="in8", bufs=N_MAIN))
    in4_pool = ctx.enter_context(tc.tile_pool(name="in4", bufs=2))
    inP_pool = ctx.enter_context(tc.tile_pool(name="inP", bufs=1))
    sS_pool = ctx.enter_context(tc.tile_pool(name="sS", bufs=3))
    tS_pool = ctx.enter_context(tc.tile_pool(name="tS", bufs=3))
    red_pool = ctx.enter_context(tc.tile_pool(name="red", bufs=2))
    acc_pool = ctx.enter_context(tc.tile_pool(name="acc", bufs=1))
    psA_pool = ctx.enter_context(tc.tile_pool(name="psA", bufs=3, space="PSUM"))
    psT_pool = ctx.enter_context(tc.tile_pool(name="psT", bufs=3, space="PSUM"))
    ps2_pool = ctx.enter_context(tc.tile_pool(name="ps2", bufs=2, space="PSUM"))

    # constants: one packed blob, one HWDGE DMA (fewer exit-drain sem
    # waits and less Sync-queue startup time); tiles are views into it.
    nco = SHED_ROWS * 64
    npo = nco + 256 + 16
    npe = npo + SHED_ROWS * 8 * P_IMG
    blob = const_pool.tile([128, npe + 16 + 128], BF16)
    nc.sync.dma_start(blob[:], consts_ap)
    wa17 = blob[:, 0:nco].rearrange("p (r i) -> p r i", r=SHED_ROWS)
    wa4 = blob[:, nco:nco + 256].rearrange("p (r i) -> p r i", r=8)
    wa1 = blob[:, nco + 256:npo].rearrange("p (r i) -> p r i", r=2)
    wap = blob[:, npo:npe].rearrange("p (r i) -> p r i", r=SHED_ROWS)
    cwt = blob[:, npe:npe + 16].rearrange("p (c j) -> p c j", c=2)
    ident = blob[:, npe + 16:npe + 144]

    acc = acc_pool.tile([8, 1], F32)
    nc.vector.memset(acc[:], 0.0)

    # P mini-spray: the HWDGE PDMA2D expansion splits a dst over the
    # LARGEST DIVISOR of its partition count <= 16 engines, starting at
    # engine 0 (measured: 28 -> engines 0-13 x2 chunks, 4 -> engines 0-3).
    # Two images routed [0:28]+[28:32] never touch slow engine 15 (~21
    # GB/s read vs ~26), trimming its end-to-end-saturated stream share by
    # ~32 KiB (~1.5 us) while the extra ~16-48 KiB rides engines with
    # >= 8 us of idle margin.  Partitions 32:128 carry zero weights; their
    # rows are memset (32-aligned offsets only -- a compute-engine rule)
    # so the cast below reads finite values.
    ptf = inP_pool.tile([128, SHED_ROWS, 256], F32, tag="ptf")
    flp = delta_ap[0:P_IMG].rearrange("q h w -> (q h) w")
    for pa, pb in ((0, 28), (28, 32)):
        nc.sync.dma_start(
            ptf[pa:pb, :, :],
            flp[pa * SHED_ROWS:pb * SHED_ROWS].rearrange(
                "(p r) w -> p (r w)", p=pb - pa, r=SHED_ROWS
            ),
        )
    nc.vector.memset(ptf[32:64, :, :], 0.0)
    nc.vector.memset(ptf[64:128, :, :], 0.0)

    # issue ALL main input DMAs upfront (SWDGE, f32->bf16 inline cast).
    subs = []
    # Each group is TWO full-128 DMAs writing disjoint row ranges of the
    # same tile (head rows + last 2 rows).  Tile's subtile deps then gate
    # only the final <=2 matmuls of stage A on the second sem: when group
    # sems complete bunched at stream end (the slow-engine-15 environment),
    # the post-sem serial chain per group shrinks from 16 matmuls to 2.
    for g in range(N_MAIN):
        gt = in8_pool.tile([128, SHED_ROWS, 256], BF16, tag="gt8")
        flv = delta_ap[P_IMG + GRP * g:P_IMG + GRP * g + GRP].rearrange(
            "q h w -> (q h) w"
        ).rearrange("(p r) w -> p r w", p=128, r=SHED_ROWS)
        nc.gpsimd.dma_start(gt[:, 0:14, :], flv[:, 0:14, :])
        nc.gpsimd.dma_start(gt[:, 14:16, :], flv[:, 14:16, :])
        subs.append((gt, "main", GRP))
    subs.append((ptf, "pmain", P_IMG))
    gt4 = in4_pool.tile([128, 8, 256], BF16, tag="gt4")
    srcv = delta_ap[90:94].rearrange("q (pp r) w -> (q pp) r w", pp=32, r=8)
    nc.gpsimd.dma_start(gt4[:, 0:6, :], srcv[:, 0:6, :])
    nc.gpsimd.dma_start(gt4[:, 6:8, :], srcv[:, 6:8, :])
    subs.append((gt4, "tail", 4))
    for t in range(2):
        gt1 = in4_pool.tile([128, 2, 256], BF16, tag="gt1")
        srcv = delta_ap[94 + t:95 + t].rearrange(
            "q (pp r) w -> (q pp) r w", pp=128, r=2
        )
        nc.gpsimd.dma_start(gt1[:, 0:1, :], srcv[:, 0:1, :])
        nc.gpsimd.dma_start(gt1[:, 1:2, :], srcv[:, 1:2, :])
        subs.append((gt1, "tail", 1))

    def stage_a(sub):
        gt, kind, n_img = sub
        if kind == "pmain":
            # the P tile arrived as raw f32 (HWDGE cannot cast): cast it
            # on-chip, split DVE/ACT ~by their elem rates.
            ptb = inP_pool.tile([128, SHED_ROWS, 256], BF16, tag="ptb")
            nc.vector.tensor_copy(ptb[:, 0:10, :], gt[:, 0:10, :])
            nc.scalar.copy(ptb[:, 10:SHED_ROWS, :], gt[:, 10:SHED_ROWS, :])
            gt = ptb
        n_out = 8 * n_img
        psumA = psA_pool.tile([n_out, 256], F32, tag="psA")
        wA = {8: wa17, 4: wa4, 2: wap, 1: wa1}[n_img]
        rows = gt.shape[1]
        for r in range(rows):
            nc.tensor.matmul(
                psumA[:], lhsT=wA[:, r, :], rhs=gt[:, r, :],
                start=(r == 0), stop=(r == rows - 1),
            )
        # PSUM -> SBUF with f32->bf16 cast (ACT engine; off the PE
        # timeline).  Two half tiles so stage B's first transpose only
        # waits on the first half (shortens the post-stream tail).
        sAs = []
        for wc in range(2):
            sA = sS_pool.tile([n_out, 128], BF16, tag=f"sA{wc}")
            nc.scalar.copy(sA[:], psumA[:, 128 * wc:128 * wc + 128])
            sAs.append(sA)
        return sAs, n_out

    def stage_b_front(sAs, n_out, tS=None, off=0):
        # 2 PE transposes (own PSUM tiles: transpose-mode output must start
        # at a bank boundary on HW) + DVE copies out.  Tail groups copy into
        # free-axis offsets of a SHARED tS so one combined back-half covers
        # all of them (shorter post-stream serial chain).
        if tS is None:
            tS = tS_pool.tile([128, 2, n_out], BF16, tag="tS")
        for wc in range(2):
            tp = psT_pool.tile([128, n_out], BF16, tag="tp")
            nc.tensor.transpose(
                tp[:],
                sAs[wc][:],
                ident[0:n_out, 0:n_out],
            )
            nc.vector.tensor_copy(tS[:, wc, off:off + n_out], tp[:])
        return tS

    def stage_b_back(tS, n_out, accumulate=True):
        # contract w into ps2[j, (q,i)], fused |.|+sum, accumulate.
        ps2 = ps2_pool.tile([8, n_out], F32, tag="ps2")
        for wc in range(2):
            nc.tensor.matmul(
                ps2[:],
                lhsT=cwt[:, wc, :],
                rhs=tS[:, wc, 0:n_out],
                start=(wc == 0),
                stop=(wc == 1),
            )
        red = red_pool.tile([8, 1], F32)
        nc.vector.tensor_reduce(
            red[:], ps2[:], axis=mybir.AxisListType.X,
            op=mybir.AluOpType.add, apply_absolute_value=True,
        )
        if accumulate:
            nc.vector.tensor_add(acc[:], acc[:], red[:])
        return red

    # Software pipeline: emit stage B of group g-1 AFTER stage A of group g,
    # so the PE (which executes in program order) never stalls mid-stream on
    # the ACT/DVE round-trips of stage B.  The tails share one tS (free-axis
    # offsets 0/32/48) and ONE combined back-half.
    tSc = tS_pool.tile([128, 2, 48], BF16, tag="tSc")
    state = {"off": 0}

    def flush(prev):
        sAs, n_out, kind = prev
        if kind in ("main", "pmain"):
            tS = stage_b_front(sAs, n_out)
            stage_b_back(tS, n_out)
        else:
            stage_b_front(sAs, n_out, tS=tSc, off=state["off"])
            state["off"] += n_out

    prev = None
    for sub in subs:
        sAs, n_out = stage_a(sub)
        cur = (sAs, n_out, sub[1])
        if prev is not None:
            flush(prev)
        prev = cur
    flush(prev)
    # acc holds the main groups; ship it while the tails finish, and ship
    # the tails' combined unaccumulated reduction separately so the final
    # out-DMA depends only on the last reduce (skips one DVE add).
    nc.sync.dma_start(out_ap[:, 0:1], acc[:])
    last_red = stage_b_back(tSc, 48, accumulate=False)
    nc.sync.dma_start(out_ap[:, 1:2], last_red[:])


# ---------------------------------------------------------------------------
# Build + run.
# ---------------------------------------------------------------------------

_CACHED_NC = None


def _build(for_sim=False):
    global _CACHED_NC, _USE_STOCK_TAIL
    if not for_sim and _CACHED_NC is not None:
        return _CACHED_NC
    _USE_STOCK_TAIL = for_sim
    nc = bass.Bass("TRN2", target_bir_lowering=False, debug=False)
    delta = nc.dram_tensor("delta", [IMGS_PER_CORE, H, W], F32, kind="ExternalInput")
    consts = nc.dram_tensor("consts", list(CONSTS.shape), BF16, kind="ExternalInput")
    out = nc.dram_tensor("out", [8, 2], F32, kind="ExternalOutput")

    with tile.TileContext(nc) as tc:
        _lowfreq_kernel(tc, out.ap(), delta.ap(), consts.ap())
    _USE_STOCK_TAIL = False
    if for_sim:
        return nc
    _strip_main_barrier(nc)
    _split_multi_waits(nc)
    _CACHED_NC = nc
    return nc


def _run(delta, **spmd_kwargs):
    import os
    os.environ["JAX_PLATFORMS"] = "axon"   # harness may have pinned cpu for the reference
    nc = _build()
    delta = np.ascontiguousarray(np.asarray(delta, dtype=np.float32))
    assert delta.shape == (B, C, H, W)
    shards = delta.reshape(N_CORES, IMGS_PER_CORE, H, W)
    in_maps = [
        {
            "delta": shards[i],
            "consts": CONSTS,
        }
        for i in range(N_CORES)
    ]
    try:
        res = bass_utils.run_bass_kernel_spmd(
            nc, in_maps, core_ids=list(range(N_CORES)), **spmd_kwargs
        )
    except Exception:
        # transient NRT_EXEC_UNIT_UNRECOVERABLE has been observed on this
        # terminal; one retry typically succeeds.
        res = bass_utils.run_bass_kernel_spmd(
            nc, in_maps, core_ids=list(range(N_CORES)), **spmd_kwargs
        )
    total = np.float64(0.0)
    for r in res.results:
        total += np.asarray(r["out"], np.float64).sum()
    return np.float32(total / TOTAL_LOW).reshape(()), res


def kernel(delta):
    out, _ = _run(delta)
    return out

